# revision 2
# baseline (speedup 1.0000x reference)
"""CRF NLL (CRFForPreTraining) on 8 Trainium2 NeuronCores.

Strategy
--------
loss = -(sum_b num_b - sum_b logZ_b).

* Numerator (gold-path score): O(B*T) gathers — computed on host in float64.
* log-partition Z: forward algorithm in exp space. With E = exp(transitions)
  and x_t = exp(em_t - BIAS), the recursion is P_{t+1} = (E^T @ P_t) ** x_t
  (** = elementwise) — one stationary [128,128] matmul (PE) plus one
  elementwise multiply (DVE) per step, state kept transposed
  [L=128 partitions, batch free].

* Time-striping: the recursion is a product of strongly mixing positive
  matrices (E is within +-10% of all-ones), so the state direction forgets
  its initial condition at ~0.1-0.25x per step. Each sequence's 1022 steps
  are split into C=16 stripes processed CONCURRENTLY as independent columns;
  each stripe (except the first) starts W=16 steps early from a uniform
  vector — by its body start the direction has converged to the true forward
  vector far below fp32 ulp. Per-column renormalization records (sigma) let
  the host stitch the per-stripe log-magnitudes exactly (float64). This
  turns 1022 serial slots into 80, with 512-wide tiles that amortize the
  fixed per-instruction costs (validated in validate_stripes.py: per-seq
  logZ error ~9e-6 = fp32 arithmetic floor).

* Data parallel over batch: 32 sequences per core x 8 cores; scalar loss
  reduced on host.
"""

import numpy as np

import concourse.bass as bass
import concourse.mybir as mybir
import concourse.tile as ctile
from concourse.tile import TileContext
from concourse.vector_clock import ScopedClock
from concourse.bass_utils import run_bass_kernel_spmd

F32 = mybir.dt.float32

B, S, L = 256, 1024, 128
NCORES = 8
BLOC = B // NCORES          # 32 sequences per core
T = S - 1                   # 1023 emission positions after CLS strip
BIAS = 5.5                  # x_t = exp(em_t - BIAS); keeps P in fp32 range

C = 32                      # time stripes per sequence
W = 8                       # warmup slots (direction convergence)
LB = 32                     # body length, stripes 1..C-1
LB0 = T - (C - 1) * LB      # stripe 0 body length = 63
NS = W + LB                 # 80 slots
NCOL = C * BLOC             # 512 state columns per core
NG = 2                      # groups for PE/DVE overlap
CG = NCOL // NG             # 256 columns per group
# measurement slots: colsums recorded after the slot's multiply (no divide —
# the e^-BIAS prescale keeps the short 40-slot chains in fp32/bf16 range)
REN = sorted({W - 1, LB0 - 1, NS - 1})
NREN = len(REN)             # 3
CHS = 8                     # slots per emission DMA chunk

# ---------------------------------------------------------------------------
# This neuronxcc build rejects instructions carrying more than one sem wait;
# TileContext's tail drain accumulates one wait per ticked proc. Split the
# surplus waits across consecutive drains on the same (SP) queue — they run
# in order before the end barrier, so semantics are unchanged.
_MAX_WAITS = 1


def _split_drain_and_barrier(self, tick_clock, wait_clock):
    nc = self.nc
    drain_inst = nc.sync.drain()
    wait_clock.add_sem_waits(
        drain_inst.ins, ScopedClock({None: tick_clock.global_clock})
    )
    si = drain_inst.ins.sync_info
    if si is not None and len(si.on_wait) > _MAX_WAITS:
        waits = list(si.on_wait)
        drain_inst.ins.sync_info = mybir.SyncInfo(
            on_wait=waits[:_MAX_WAITS], on_update=list(si.on_update)
        )
        for i in range(_MAX_WAITS, len(waits), _MAX_WAITS):
            extra = nc.sync.drain()
            extra.ins.sync_info = mybir.SyncInfo(
                on_wait=waits[i : i + _MAX_WAITS], on_update=[]
            )
    nc.all_engine_barrier()
    assert self.sems is not None
    popped = nc._tile_sem_poison_stack.pop()
    assert popped is self._sem_poison
    nc.clear_and_free_semaphores(list(self.sems.allocated().values()))
    nc.all_engine_barrier()


ctile.TileContext._drain_and_barrier = _split_drain_and_barrier


def _split_multi_waits_json(raw: bytes) -> bytes:
    """Rewrite BIR JSON so no instruction carries more than one sem wait.

    Engine queues execute in order, so an instruction's surplus waits can be
    moved onto NoOps inserted immediately before it on the same queue.
    """
    import json

    mod = json.loads(raw)
    for fn in mod["functions"]:
        for bb in fn["blocks"]:
            out = []
            for inst in bb["instructions"]:
                si = inst.get("sync_info") or {}
                ow = si.get("on_wait") or []
                if len(ow) > 1:
                    for i, w in enumerate(ow[:-1]):
                        out.append(
                            {
                                "debug": inst.get("debug", 0),
                                "engine": inst["engine"],
                                "ins": [],
                                "name": f"{inst['name']}_w{i}",
                                "opcode": "NoOp",
                                "outs": [],
                                "sync_info": {"on_update": [], "on_wait": [w]},
                            }
                        )
                    si = dict(si)
                    si["on_wait"] = [ow[-1]]
                    inst = {**inst, "sync_info": si}
                out.append(inst)
            bb["instructions"] = out
    return json.dumps(mod).encode()


# ---------------------------------------------------------------------------
def _build_nc(repeat: int = 1, bf16: bool = False, xbf16: bool = False) -> bass.Bass:
    DT = mybir.dt.bfloat16 if bf16 else F32
    XDT = mybir.dt.bfloat16 if xbf16 else F32
    nc = bass.Bass()
    xin = nc.dram_tensor("xin", [L, NS * NCOL], XDT, kind="ExternalInput")
    trans = nc.dram_tensor("trans", [L, L], F32, kind="ExternalInput")
    startv = nc.dram_tensor("startv", [L, 1], F32, kind="ExternalInput")
    endv = nc.dram_tensor("endv", [L, 1], F32, kind="ExternalInput")
    pe_out = nc.dram_tensor("pe", [repeat * BLOC, 1], F32, kind="ExternalOutput")
    sv_out = nc.dram_tensor("sv", [repeat, NREN * NCOL], F32, kind="ExternalOutput")

    with TileContext(nc) as tc:
        with (
            tc.tile_pool(name="const", bufs=1) as cpool,
            tc.tile_pool(name="x", bufs=3) as xpool,
            tc.tile_pool(name="p", bufs=4) as ppool,
            tc.tile_pool(name="sv", bufs=1) as svpool,
            tc.tile_pool(name="ps", bufs=2, space="PSUM") as pspool,
            tc.tile_pool(name="ps2", bufs=1, space="PSUM") as ps2pool,
        ):
            # ---- constants -------------------------------------------------
            traw = cpool.tile([L, L], F32)
            nc.sync.dma_start(traw[:], trans[:])
            E = cpool.tile([L, L], DT)
            nc.scalar.activation(E[:], traw[:], mybir.ActivationFunctionType.Exp)

            sraw = cpool.tile([L, 1], F32)
            nc.sync.dma_start(sraw[:], startv[:])
            expstart = cpool.tile([L, 1], F32)
            nc.scalar.activation(
                expstart[:], sraw[:], mybir.ActivationFunctionType.Exp
            )

            eraw = cpool.tile([L, 1], F32)
            nc.sync.dma_start(eraw[:], endv[:])
            expend = cpool.tile([L, 1], F32)
            nc.scalar.activation(expend[:], eraw[:], mybir.ActivationFunctionType.Exp)

            allones = cpool.tile([L, L], F32)
            nc.gpsimd.memset(allones[:], 1.0)

            allones_dt = cpool.tile([L, L], DT)
            nc.gpsimd.memset(allones_dt[:], 1.0)

            sv = svpool.tile([1, NREN * NCOL], F32)

            for _rep in range(repeat):
                # ---- chunked emission load + exp ---------------------------
                nchunks = (NS + CHS - 1) // CHS
                xtiles = []
                for ci in range(nchunks):
                    s0 = ci * CHS
                    ln = min(CHS, NS - s0)
                    xc = xpool.tile([L, ln * NCOL], XDT, tag="x")
                    nc.sync.dma_start(
                        xc[:], xin[:, s0 * NCOL : (s0 + ln) * NCOL]
                    )
                    xtiles.append((s0, xc))

                def xslice(tau, g):
                    s0, xc = xtiles[tau // CHS]
                    off = (tau - s0) * NCOL + g * CG
                    return xc[:, off : off + CG]

                # ---- init (slot 0) -----------------------------------------
                # group 0 columns 0:32 are stripe 0 -> seed with exp(start);
                # all other columns seed with the raw x (uniform warmup).
                P = []
                for g in range(NG):
                    p0 = ppool.tile([L, CG], DT, tag=f"p{g}")
                    if g == 0:
                        nc.vector.tensor_scalar_mul(
                            p0[:, 0:BLOC], xslice(0, 0)[:, 0:BLOC], expstart[:]
                        )
                        nc.vector.tensor_copy(
                            p0[:, BLOC:CG], xslice(0, 0)[:, BLOC:CG]
                        )
                    else:
                        nc.vector.tensor_copy(p0[:], xslice(0, g))
                    P.append(p0)

                # ---- slots 1..NS-1 -----------------------------------------
                rev = 0
                for tau in range(1, NS):
                    renorm = tau in REN
                    for g in range(NG):
                        ps = pspool.tile([L, CG], F32, tag=f"ps{g}")
                        nc.tensor.matmul(ps[:], E[:], P[g][:])
                        pn = ppool.tile([L, CG], DT, tag=f"p{g}")
                        nc.vector.tensor_mul(pn[:], ps[:], xslice(tau, g))
                        P[g] = pn
                        if renorm:
                            sg = ps2pool.tile([L, CG], F32, tag=f"sg{g}")
                            nc.tensor.matmul(sg[:], allones_dt[:, :L], pn[:])
                            col = rev * NCOL + g * CG
                            nc.vector.tensor_copy(
                                sv[0:1, col : col + CG], sg[0:1, :]
                            )
                    if renorm:
                        rev += 1
                rev_total = rev

                # ---- epilogue: e-weighted sum for last stripe's columns ----
                y = ppool.tile([L, BLOC], F32, tag="y")
                nc.vector.tensor_scalar_mul(
                    y[:], P[NG - 1][:, CG - BLOC : CG], expend[:]
                )
                pf = ps2pool.tile([BLOC, 1], F32, tag="pf")
                nc.tensor.matmul(pf[:], y[:], allones[:, 0:1])
                pfs = ppool.tile([BLOC, 1], F32, tag="pfs")
                nc.scalar.copy(pfs[:], pf[:])
                nc.sync.dma_start(
                    pe_out[_rep * BLOC : (_rep + 1) * BLOC, :], pfs[:]
                )
                nc.sync.dma_start(sv_out[_rep : _rep + 1, :], sv[:])
                rev = 0

            assert rev_total == NREN

    return nc


TRACE = False        # unused here (no NTFF hook in this env); kept for parity
LAST_RESULT = None   # BassKernelResults of the most recent run

_NC_CACHE: dict[tuple, bass.Bass] = {}


def _get_nc(repeat: int = 1, bf16: bool | None = None,
            xbf16: bool | None = None) -> bass.Bass:
    if bf16 is None:
        bf16 = BF16
    if xbf16 is None:
        xbf16 = XBF16
    key = (repeat, bf16, xbf16)
    if key not in _NC_CACHE:
        nc = _build_nc(repeat, bf16, xbf16)
        orig = nc.to_json_bytes
        nc.to_json_bytes = lambda *a, **k: _split_multi_waits_json(orig(*a, **k))
        _NC_CACHE[key] = nc
    return _NC_CACHE[key]


# ---------------------------------------------------------------------------
def _numerator_host(emissions, labels, mask, start_t, end_t, trans):
    """Gold-path score per sequence, float64. [B]"""
    em = emissions[:, 1:, :]
    tags = labels[:, 1:].astype(np.int64)
    m = mask[:, 1:].astype(bool)
    mf = m.astype(np.float64)
    emit = np.take_along_axis(em, tags[..., None], axis=2)[..., 0].astype(np.float64)
    num = start_t.astype(np.float64)[tags[:, 0]] + emit[:, 0]
    tr = trans.astype(np.float64)
    num = num + (tr[tags[:, :-1], tags[:, 1:]] * mf[:, 1:]).sum(axis=1)
    num = num + (emit[:, 1:] * mf[:, 1:]).sum(axis=1)
    seq_ends = m.sum(axis=1).astype(np.int64) - 1
    last_tags = np.take_along_axis(tags, seq_ends[:, None], axis=1)[:, 0]
    num = num + end_t.astype(np.float64)[last_tags]
    return num


def _crf_nll_numpy(emissions, labels, mask, start_t, end_t, trans):
    """Full float64 fallback (only used if mask has zeros)."""
    num = _numerator_host(emissions, labels, mask, start_t, end_t, trans)
    em = emissions[:, 1:, :].astype(np.float64)
    m = mask[:, 1:].astype(bool)
    alpha = start_t.astype(np.float64)[None, :] + em[:, 0]
    tr = trans.astype(np.float64)
    for t in range(1, em.shape[1]):
        mx = alpha.max(axis=1, keepdims=True)
        nxt = mx + np.log(np.exp(alpha - mx) @ np.exp(tr)) + em[:, t]
        alpha = np.where(m[:, t][:, None], nxt, alpha)
    mx = alpha.max(axis=1)
    logz = mx + np.log(
        np.exp(alpha - mx[:, None] + end_t.astype(np.float64)[None, :]).sum(axis=1)
    )
    return -(num - logz).sum()


def _build_xin(em_core, em_pad):
    """[BLOC, T, L] core emissions -> strided x = exp(em - BIAS) slot layout.

    Column order within a slot: (stripe, b). Stripe 0 runs t = 0.. with no
    warmup and pads after its body; stripe s>=1 covers
    t in [tstart_s, tstart_s + NS) with tstart_s = LB0 + LB*(s-1) - W.
    """
    emT = em_core.transpose(2, 1, 0)                      # [L, T, BLOC]
    out = np.empty((L, NS, C, BLOC), dtype=np.float32)
    # stripe 0
    out[:, :LB0, 0, :] = emT[:, :LB0, :]
    out[:, LB0:, 0, :] = em_pad
    # stripes 1..C-1
    for s in range(1, C):
        t0 = LB0 + LB * (s - 1) - W
        out[:, :, s, :] = emT[:, t0 : t0 + NS, :]
    out = np.exp(out.reshape(L, NS * NCOL) - np.float32(BIAS))
    if XBF16:
        import ml_dtypes

        return np.ascontiguousarray(out.astype(ml_dtypes.bfloat16))
    return np.ascontiguousarray(out)


# ---------------------------------------------------------------------------
def build_in_maps(emissions, start_t, end_t, trans):
    """Full [B,S,L] emissions + params -> per-core input dicts."""
    em_pad = np.float32(BIAS - np.log(np.exp(trans.astype(np.float64)).mean() * L))
    common = {
        "trans": np.ascontiguousarray(trans),
        "startv": np.ascontiguousarray(start_t[:, None]),
        "endv": np.ascontiguousarray(end_t[:, None]),
    }
    em = emissions[:, 1:, :]  # [B, T, L]
    in_maps = []
    for c in range(NCORES):
        em_c = em[c * BLOC : (c + 1) * BLOC]
        in_maps.append({"xin": _build_xin(em_c, em_pad), **common})
    return in_maps


def stitch_loss(per_core_results, num, rep=0):
    """Host stitch (float64) of device colsum records -> scalar loss.

    sv[e, col]: colsum snapshots at slot REN[e] (no on-device divides).
    e0 = warmup end (W-1), e1 = stripe-0 body end (LB0-1), e2 = last slot.
    stripe 0:        ln cs(e1) + BIAS*LB0
    stripes 1..C-2:  ln cs(e2) - ln cs(e0) + BIAS*LB
    stripe C-1:      ln pe     - ln cs(e0) + BIAS*LB
    """
    logz = np.empty(B, dtype=np.float64)
    for c in range(NCORES):
        sv = per_core_results[c]["sv"][rep].reshape(NREN, C, BLOC).astype(np.float64)
        pe = per_core_results[c]["pe"][rep * BLOC : (rep + 1) * BLOC, 0].astype(
            np.float64
        )
        lsv = np.log(sv)
        lz = np.empty((C, BLOC), dtype=np.float64)
        lz[0] = lsv[1, 0, :] + BIAS * LB0
        lz[1:-1] = lsv[2, 1:-1, :] - lsv[0, 1:-1, :] + BIAS * LB
        lz[-1] = np.log(pe) - lsv[0, -1, :] + BIAS * LB
        logz[c * BLOC : (c + 1) * BLOC] = lz.sum(axis=0)
    return -(num - logz).sum()


def kernel(emissions, labels, mask, start_transitions, end_transitions,
           transitions):
    emissions = np.asarray(emissions, dtype=np.float32)
    labels = np.asarray(labels)
    mask = np.asarray(mask).astype(bool)
    start_t = np.asarray(start_transitions, dtype=np.float32)
    end_t = np.asarray(end_transitions, dtype=np.float32)
    trans = np.asarray(transitions, dtype=np.float32)

    if emissions.shape != (B, S, L) or not mask[:, 1:].all():
        return np.float32(
            _crf_nll_numpy(emissions, labels, mask, start_t, end_t, trans)
        )

    num = _numerator_host(emissions, labels, mask, start_t, end_t, trans)

    # ---- device: logZ ------------------------------------------------------
    nc = _get_nc(REPEAT)
    in_maps = build_in_maps(emissions, start_t, end_t, trans)

    global LAST_RESULT
    res = run_bass_kernel_spmd(nc, in_maps, core_ids=list(range(NCORES)))
    LAST_RESULT = res

    loss = stitch_loss(res.results, num)
    return np.float32(loss)


REPEAT = 1
BF16 = True
XBF16 = True

